# revision 14
# baseline (speedup 1.0000x reference)
"""Multi-head attention block (QKV proj + softmax attention + out proj +
residual + LayerNorm) on 8 Trainium2 NeuronCores.

Sharding:
  Phase A: head-parallel — core c computes heads (2c, 2c+1) for both batch
           elements: Q/K/V projections, scores (transposed layout), exp,
           unnormalized P@V and softmax denominators.
  Phase B: row-parallel — core c computes 512 rows of the flattened (B*L)
           output: per-head normalization, output projection (contracting
           over all 16 heads), residual add and LayerNorm.

Matmul operands in fp16 (PE streams 2-byte operands at full rate); all
accumulation in fp32 PSUM; softmax/LayerNorm arithmetic in fp32.
exp uses a fixed -2.0 bias to keep fp16 P values in range; it cancels in
the softmax normalization since denominators use the same biased values.
"""

import sys

if "/opt/trn_rl_repo" not in sys.path:
    sys.path.insert(0, "/opt/trn_rl_repo")

import ml_dtypes
import numpy as np

import concourse.bass as bass
import concourse.tile as tile
from concourse import bacc, mybir
from concourse.bass_utils import run_bass_kernel_spmd
from concourse.masks import make_identity

B, L, D, H, DQ = 2, 2048, 1024, 16, 64
N_CORES = 8
LN_EPS = 1e-5
F32 = mybir.dt.float32
FP16 = mybir.dt.float16
AF = mybir.ActivationFunctionType
OP = mybir.AluOpType
FP16_NP = np.float16

_cache = {}


def _build_phase_a():
    nc = bacc.Bacc("TRN2", target_bir_lowering=False, debug=False, num_devices=N_CORES)
    xt_d = nc.dram_tensor("xt", [B, D, L], FP16, kind="ExternalInput").ap()
    wq_d = nc.dram_tensor("wq", [D, 128], FP16, kind="ExternalInput").ap()
    wk_d = nc.dram_tensor("wk", [D, 128], FP16, kind="ExternalInput").ap()
    wv_d = nc.dram_tensor("wv", [D, 128], FP16, kind="ExternalInput").ap()
    # rows 0..127: A^T (2 heads x 64), rows 128..129: softmax denominators
    at_d = nc.dram_tensor("at", [B, 130, L], F32, kind="ExternalOutput").ap()

    with tile.TileContext(nc) as tc:
        with tc.tile_pool(name="singles", bufs=1) as singles, \
             tc.tile_pool(name="proj_sb", bufs=2) as proj_sb, \
             tc.tile_pool(name="pt_sb", bufs=4) as pt_sb, \
             tc.tile_pool(name="out_sb", bufs=3) as out_sb, \
             tc.tile_pool(name="ps_mm", bufs=3, space="PSUM") as ps_mm, \
             tc.tile_pool(name="ps_apv", bufs=2, space="PSUM") as ps_apv:
            w_sb = {}
            for nm, d in (("wq", wq_d), ("wk", wk_d), ("wv", wv_d)):
                t = singles.tile([128, 8, 128], FP16, tag=nm)
                nc.sync.dma_start(out=t, in_=d.rearrange("(mc p) h -> p mc h", p=128))
                w_sb[nm] = t
            ident_f = singles.tile([128, 128], F32, tag="ident_f")
            make_identity(nc, ident_f)
            ident = singles.tile([128, 128], FP16, tag="ident")
            nc.vector.tensor_copy(out=ident, in_=ident_f)
            exp_bias = singles.tile([128, 1], F32, tag="exp_bias")
            nc.vector.memset(exp_bias, -8.0)

            for b in range(B):
                xt_b = singles.tile([128, 8, L], FP16, tag="xt")
                nc.sync.dma_start(
                    out=xt_b, in_=xt_d[b].rearrange("(mc p) l -> p mc l", p=128)
                )
                qt = proj_sb.tile([128, L], FP16, tag="qt")
                kt = proj_sb.tile([128, L], FP16, tag="kt")
                vt = proj_sb.tile([128, L], FP16, tag="vt")
                for dst, w in ((qt, w_sb["wq"]), (kt, w_sb["wk"]), (vt, w_sb["wv"])):
                    for it in range(4):
                        sl = slice(it * 512, (it + 1) * 512)
                        ps = ps_mm.tile([128, 512], F32, tag="mm")
                        for mc in range(8):
                            nc.tensor.matmul(
                                ps, w[:, mc, :], xt_b[:, mc, sl],
                                start=(mc == 0), stop=(mc == 7),
                            )
                        nc.vector.tensor_copy(out=dst[:, sl], in_=ps)
                # V in natural [j, q] layout per head, with a ones column at 64
                # (gives softmax denominators as row 64 of the PV product)
                vsb = proj_sb.tile([128, 2, 16, 65], FP16, tag="vsb")
                ones_f = singles.tile([128, 2, 16, 1], FP16, tag="ones")
                nc.vector.memset(ones_f, 1.0)
                nc.vector.tensor_copy(out=vsb[:, :, :, 64:65], in_=ones_f)
                for jc in range(16):
                    for h in range(2):
                        ps = ps_mm.tile([128, 512], FP16, tag="mm")
                        nc.tensor.transpose(
                            ps[:, 0:64],
                            vt[h * 64:(h + 1) * 64, jc * 128:(jc + 1) * 128],
                            ident[h * 64:(h + 1) * 64, h * 64:(h + 1) * 64],
                        )
                        nc.vector.tensor_copy(out=vsb[:, h, jc, 0:64], in_=ps[:, 0:64])

                for it in range(4):
                    i_sl = slice(it * 512, (it + 1) * 512)
                    apv = [
                        ps_apv.tile([65, 512], F32, tag="apv", name=f"apv{_h}")
                        for _h in range(2)
                    ]
                    for g in range(8):
                        st = [
                            ps_mm.tile([128, 1024], F32, tag="mm", name=f"st{_h}")
                            for _h in range(2)
                        ]
                        # adjacent h-pairs at row positions 0/64 run
                        # concurrently in the PE array (K=64 each)
                        for s in range(2):
                            jc = 2 * g + s
                            for h in range(2):
                                hs = slice(h * 64, (h + 1) * 64)
                                nc.tensor.matmul(
                                    st[h][:, s * 512:(s + 1) * 512],
                                    kt[hs, jc * 128:(jc + 1) * 128],
                                    qt[hs, i_sl],
                                    start=True, stop=True,
                                )
                        ptt = [
                            pt_sb.tile([128, 1024], FP16, tag="pt", name=f"pt{_h}")
                            for _h in range(2)
                        ]
                        for h in range(2):
                            nc.scalar.activation(
                                out=ptt[h], in_=st[h], func=AF.Exp,
                                scale=1.0 / (DQ ** 0.5), bias=exp_bias,
                            )
                        for s in range(2):
                            jc = 2 * g + s
                            for h in range(2):
                                nc.tensor.matmul(
                                    apv[h],
                                    vsb[:, h, jc, :],
                                    ptt[h][:, s * 512:(s + 1) * 512],
                                    start=(g == 0 and s == 0),
                                    stop=(g == 7 and s == 1),
                                )
                    for h in range(2):
                        o_sb = out_sb.tile([65, 512], F32, tag="o")
                        nc.vector.tensor_copy(out=o_sb, in_=apv[h])
                        nc.sync.dma_start(
                            out=at_d[b, h * 64:(h + 1) * 64, i_sl], in_=o_sb[0:64, :]
                        )
                        nc.sync.dma_start(
                            out=at_d[b, 128 + h:129 + h, i_sl], in_=o_sb[64:65, :]
                        )
    nc.compile()
    return nc


def _build_phase_b():
    nc = bacc.Bacc("TRN2", target_bir_lowering=False, debug=False, num_devices=N_CORES)
    ROWS = B * L // N_CORES  # 512
    atq_d = nc.dram_tensor("atq", [H * DQ, ROWS], FP16, kind="ExternalInput").ap()
    rdn_d = nc.dram_tensor("rdn", [H * DQ, ROWS], FP16, kind="ExternalInput").ap()
    xr_d = nc.dram_tensor("xr", [ROWS, D], F32, kind="ExternalInput").ap()
    wo_d = nc.dram_tensor("wo", [H * DQ, D], FP16, kind="ExternalInput").ap()
    g_d = nc.dram_tensor("gamma", [D], F32, kind="ExternalInput").ap()
    bt_d = nc.dram_tensor("beta", [D], F32, kind="ExternalInput").ap()
    y_d = nc.dram_tensor("y", [ROWS, D], F32, kind="ExternalOutput").ap()

    with tile.TileContext(nc) as tc:
        with tc.tile_pool(name="sb", bufs=1) as sb, \
             tc.tile_pool(name="yt_sb", bufs=3) as yt_sb, \
             tc.tile_pool(name="st_sb", bufs=4) as st_sb, \
             tc.tile_pool(name="ps", bufs=4, space="PSUM") as ps_pool:
            atq = sb.tile([128, 8, ROWS], FP16, tag="atq")
            nc.sync.dma_start(out=atq, in_=atq_d.rearrange("(hc p) i -> p hc i", p=128))
            rdn = sb.tile([128, 8, ROWS], FP16, tag="rdn")
            nc.sync.dma_start(out=rdn, in_=rdn_d.rearrange("(hc p) i -> p hc i", p=128))
            atn = sb.tile([128, 8, ROWS], FP16, tag="atn")
            nc.vector.tensor_tensor(out=atn, in0=atq, in1=rdn, op=OP.mult)
            wo = sb.tile([128, 8, D], FP16, tag="wo")
            nc.sync.dma_start(out=wo, in_=wo_d.rearrange("(hc p) m -> p hc m", p=128))
            x_sb = sb.tile([128, 4, D], F32, tag="x")
            nc.sync.dma_start(out=x_sb, in_=xr_d.rearrange("(ic p) m -> p ic m", p=128))
            gb = sb.tile([128, D], F32, tag="gb")
            nc.sync.dma_start(
                out=gb,
                in_=bass.AP(tensor=g_d.tensor, offset=g_d.offset, ap=[[0, 128]] + g_d.ap),
            )
            bb = sb.tile([128, D], F32, tag="bb")
            nc.sync.dma_start(
                out=bb,
                in_=bass.AP(tensor=bt_d.tensor, offset=bt_d.offset, ap=[[0, 128]] + bt_d.ap),
            )
            eps_t = sb.tile([128, 1], F32, tag="eps")
            nc.vector.memset(eps_t, LN_EPS)

            for ic in range(4):
                yt = yt_sb.tile([128, D], F32, tag="yt")
                for mh in range(2):
                    o_ps = ps_pool.tile([128, 512], F32, tag="o")
                    for hc in range(8):
                        nc.tensor.matmul(
                            o_ps,
                            atn[:, hc, ic * 128:(ic + 1) * 128],
                            wo[:, hc, mh * 512:(mh + 1) * 512],
                            start=(hc == 0), stop=(hc == 7),
                        )
                    nc.vector.tensor_tensor(
                        out=yt[:, mh * 512:(mh + 1) * 512],
                        in0=o_ps,
                        in1=x_sb[:, ic, mh * 512:(mh + 1) * 512],
                        op=OP.add,
                    )
                stats = st_sb.tile([128, 2, 6], F32, tag="stats")
                for sg in range(2):
                    nc.vector.bn_stats(
                        out=stats[:, sg, :], in_=yt[:, sg * 512:(sg + 1) * 512]
                    )
                mv = st_sb.tile([128, 2], F32, tag="mv")
                nc.vector.bn_aggr(out=mv, in_=stats)
                rstd = st_sb.tile([128, 1], F32, tag="rstd")
                nc.scalar.activation(
                    out=rstd, in_=mv[:, 1:2], func=AF.Sqrt, bias=eps_t, scale=1.0
                )
                nc.vector.reciprocal(out=rstd, in_=rstd)
                nc.vector.tensor_scalar(
                    out=yt, in0=yt, scalar1=mv[:, 0:1], scalar2=rstd,
                    op0=OP.subtract, op1=OP.mult,
                )
                nc.vector.tensor_tensor(out=yt, in0=yt, in1=gb, op=OP.mult)
                nc.vector.tensor_tensor(out=yt, in0=yt, in1=bb, op=OP.add)
                nc.sync.dma_start(out=y_d[ic * 128:(ic + 1) * 128, :], in_=yt)
    nc.compile()
    return nc


def _prep_a(x, w_q, w_k, w_v):
    xt = np.ascontiguousarray(x.transpose(0, 2, 1)).astype(FP16_NP)  # [B, D, L]

    def w_slice(w, c):
        return np.ascontiguousarray(
            w[2 * c:2 * c + 2].transpose(1, 0, 2).reshape(D, 2 * DQ)
        ).astype(FP16_NP)

    return [
        {
            "xt": xt,
            "wq": w_slice(w_q, c),
            "wk": w_slice(w_k, c),
            "wv": w_slice(w_v, c),
        }
        for c in range(N_CORES)
    ]


def _prep_b(res_a_results, x, w_o, ln_gamma, ln_beta):
    at_full = np.concatenate(
        [res_a_results[c]["at"][:, :128, :] for c in range(N_CORES)], axis=1
    )  # [B, H*DQ, L]
    den = np.stack(
        [res_a_results[c]["at"][:, 128:130, :] for c in range(N_CORES)], axis=1
    ).reshape(B, H, L)
    rdn_full = np.repeat((1.0 / den).astype(np.float32), DQ, axis=1)  # [B, H*DQ, L]

    ROWS = B * L // N_CORES
    wo_flat = np.ascontiguousarray(w_o.reshape(H * DQ, D)).astype(FP16_NP)
    # exact power-of-two rescale keeps both factors in fp16 range;
    # it cancels exactly in the on-device product
    at_bf = (at_full * (1.0 / 64.0)).astype(FP16_NP)
    rdn_bf = (rdn_full * 64.0).astype(FP16_NP)
    in_maps_b = []
    for c in range(N_CORES):
        b = c // (N_CORES // B)
        l0 = (c % (N_CORES // B)) * ROWS
        in_maps_b.append(
            {
                "atq": np.ascontiguousarray(at_bf[b][:, l0:l0 + ROWS]),
                "rdn": np.ascontiguousarray(rdn_bf[b][:, l0:l0 + ROWS]),
                "xr": np.ascontiguousarray(x[b, l0:l0 + ROWS]),
                "wo": wo_flat,
                "gamma": ln_gamma,
                "beta": ln_beta,
            }
        )
    return in_maps_b


def kernel(x, w_q, w_k, w_v, w_o, ln_gamma, ln_beta):
    x = np.asarray(x, dtype=np.float32)
    w_q = np.asarray(w_q, dtype=np.float32)
    w_k = np.asarray(w_k, dtype=np.float32)
    w_v = np.asarray(w_v, dtype=np.float32)
    w_o = np.asarray(w_o, dtype=np.float32)
    ln_gamma = np.asarray(ln_gamma, dtype=np.float32)
    ln_beta = np.asarray(ln_beta, dtype=np.float32)

    if "a" not in _cache:
        _cache["a"] = _build_phase_a()
    if "b" not in _cache:
        _cache["b"] = _build_phase_b()

    in_maps_a = _prep_a(x, w_q, w_k, w_v)
    res_a = run_bass_kernel_spmd(
        _cache["a"], in_maps_a, core_ids=list(range(N_CORES)), trace=False
    )
    in_maps_b = _prep_b(res_a.results, x, w_o, ln_gamma, ln_beta)
    res_b = run_bass_kernel_spmd(
        _cache["b"], in_maps_b, core_ids=list(range(N_CORES)), trace=False
    )
    y = np.concatenate([res_b.results[c]["y"] for c in range(N_CORES)], axis=0)
    return y.reshape(B, L, D)


# revision 15
# speedup vs baseline: 1.4350x; 1.4350x over previous
"""Multi-head attention block (QKV proj + softmax attention + out proj +
residual + LayerNorm) on 8 Trainium2 NeuronCores.

Sharding:
  Phase A: head-parallel — core c computes heads (2c, 2c+1) for both batch
           elements: Q/K/V projections, scores (transposed layout), exp,
           unnormalized P@V and softmax denominators.
  Phase B: row-parallel — core c computes 512 rows of the flattened (B*L)
           output: per-head normalization, output projection (contracting
           over all 16 heads), residual add and LayerNorm.

Matmul operands in fp16 (PE streams 2-byte operands at full rate); all
accumulation in fp32 PSUM; softmax/LayerNorm arithmetic in fp32.
exp uses a fixed -2.0 bias to keep fp16 P values in range; it cancels in
the softmax normalization since denominators use the same biased values.
"""

import sys

if "/opt/trn_rl_repo" not in sys.path:
    sys.path.insert(0, "/opt/trn_rl_repo")

import ml_dtypes
import numpy as np

import concourse.bass as bass
import concourse.tile as tile
from concourse import bacc, mybir
from concourse.bass_utils import run_bass_kernel_spmd
from concourse.masks import make_identity

B, L, D, H, DQ = 2, 2048, 1024, 16, 64
N_CORES = 8
LN_EPS = 1e-5
F32 = mybir.dt.float32
FP16 = mybir.dt.float16
AF = mybir.ActivationFunctionType
OP = mybir.AluOpType
FP16_NP = np.float16

_cache = {}


def _build_phase_a():
    nc = bacc.Bacc("TRN2", target_bir_lowering=False, debug=False, num_devices=N_CORES)
    xt_d = nc.dram_tensor("xt", [B, D, L], FP16, kind="ExternalInput").ap()
    wq_d = nc.dram_tensor("wq", [D, 128], FP16, kind="ExternalInput").ap()
    wk_d = nc.dram_tensor("wk", [D, 128], FP16, kind="ExternalInput").ap()
    wv_d = nc.dram_tensor("wv", [D, 128], FP16, kind="ExternalInput").ap()
    # rows 0..127: A^T (2 heads x 64), rows 128..129: softmax denominators
    at_d = nc.dram_tensor("at", [B, 130, L], F32, kind="ExternalOutput").ap()

    with tile.TileContext(nc) as tc:
        with tc.tile_pool(name="singles", bufs=1) as singles, \
             tc.tile_pool(name="proj_sb", bufs=2) as proj_sb, \
             tc.tile_pool(name="pt_sb", bufs=4) as pt_sb, \
             tc.tile_pool(name="out_sb", bufs=3) as out_sb, \
             tc.tile_pool(name="ps_mm", bufs=3, space="PSUM") as ps_mm, \
             tc.tile_pool(name="ps_apv", bufs=2, space="PSUM") as ps_apv:
            w_sb = {}
            for nm, d in (("wq", wq_d), ("wk", wk_d), ("wv", wv_d)):
                t = singles.tile([128, 8, 128], FP16, tag=nm)
                nc.sync.dma_start(out=t, in_=d.rearrange("(mc p) h -> p mc h", p=128))
                w_sb[nm] = t
            ident_f = singles.tile([128, 128], F32, tag="ident_f")
            make_identity(nc, ident_f)
            ident = singles.tile([128, 128], FP16, tag="ident")
            nc.vector.tensor_copy(out=ident, in_=ident_f)
            exp_bias = singles.tile([128, 1], F32, tag="exp_bias")
            nc.vector.memset(exp_bias, -8.0)

            for b in range(B):
                xt_b = singles.tile([128, 8, L], FP16, tag="xt")
                nc.sync.dma_start(
                    out=xt_b, in_=xt_d[b].rearrange("(mc p) l -> p mc l", p=128)
                )
                qt = proj_sb.tile([128, L], FP16, tag="qt")
                kt = proj_sb.tile([128, L], FP16, tag="kt")
                vt = proj_sb.tile([128, L], FP16, tag="vt")
                for dst, w in ((qt, w_sb["wq"]), (kt, w_sb["wk"]), (vt, w_sb["wv"])):
                    for it in range(4):
                        sl = slice(it * 512, (it + 1) * 512)
                        ps = ps_mm.tile([128, 512], F32, tag="mm")
                        for mc in range(8):
                            nc.tensor.matmul(
                                ps, w[:, mc, :], xt_b[:, mc, sl],
                                start=(mc == 0), stop=(mc == 7),
                            )
                        nc.vector.tensor_copy(out=dst[:, sl], in_=ps)
                # V in natural [j, q] layout per head, with a ones column at 64
                # (gives softmax denominators as row 64 of the PV product)
                vsb = proj_sb.tile([128, 2, 16, 128], FP16, tag="vsb")
                nc.vector.memset(vsb, 0.0)
                ones_f = singles.tile([128, 2, 16, 1], FP16, tag="ones")
                nc.vector.memset(ones_f, 1.0)
                nc.vector.tensor_copy(out=vsb[:, :, :, 64:65], in_=ones_f)
                for jc in range(16):
                    for h in range(2):
                        ps = ps_mm.tile([128, 512], FP16, tag="mm")
                        nc.tensor.transpose(
                            ps[:, 0:64],
                            vt[h * 64:(h + 1) * 64, jc * 128:(jc + 1) * 128],
                            ident[h * 64:(h + 1) * 64, h * 64:(h + 1) * 64],
                        )
                        nc.vector.tensor_copy(out=vsb[:, h, jc, 0:64], in_=ps[:, 0:64])

                for it in range(4):
                    i_sl = slice(it * 512, (it + 1) * 512)
                    apv = [
                        ps_apv.tile([128, 512], F32, tag="apv", name=f"apv{_h}")
                        for _h in range(2)
                    ]
                    for jc in range(16):
                        # one ST tile holds both heads' scores for this jc —
                        # the shared exp forces the two K=64 matmuls adjacent
                        # so they run concurrently in disjoint PE row groups
                        st = ps_mm.tile([128, 1024], F32, tag="mm")
                        for h in range(2):
                            hs = slice(h * 64, (h + 1) * 64)
                            nc.tensor.matmul(
                                st[:, h * 512:(h + 1) * 512],
                                kt[hs, jc * 128:(jc + 1) * 128],
                                qt[hs, i_sl],
                                start=True, stop=True,
                            )
                        ptt = pt_sb.tile([128, 1024], FP16, tag="pt")
                        nc.scalar.activation(
                            out=ptt, in_=st, func=AF.Exp,
                            scale=1.0 / (DQ ** 0.5), bias=exp_bias,
                        )
                        for h in range(2):
                            nc.tensor.matmul(
                                apv[h],
                                vsb[:, h, jc, :],
                                ptt[:, h * 512:(h + 1) * 512],
                                start=(jc == 0),
                                stop=(jc == 15),
                            )
                    for h in range(2):
                        o_sb = out_sb.tile([65, 512], F32, tag="o")
                        nc.vector.tensor_copy(out=o_sb, in_=apv[h][0:65, :])
                        nc.sync.dma_start(
                            out=at_d[b, h * 64:(h + 1) * 64, i_sl], in_=o_sb[0:64, :]
                        )
                        nc.sync.dma_start(
                            out=at_d[b, 128 + h:129 + h, i_sl], in_=o_sb[64:65, :]
                        )
    nc.compile()
    return nc


def _build_phase_b():
    nc = bacc.Bacc("TRN2", target_bir_lowering=False, debug=False, num_devices=N_CORES)
    ROWS = B * L // N_CORES  # 512
    atq_d = nc.dram_tensor("atq", [H * DQ, ROWS], FP16, kind="ExternalInput").ap()
    rdn_d = nc.dram_tensor("rdn", [H * DQ, ROWS], FP16, kind="ExternalInput").ap()
    xr_d = nc.dram_tensor("xr", [ROWS, D], F32, kind="ExternalInput").ap()
    wo_d = nc.dram_tensor("wo", [H * DQ, D], FP16, kind="ExternalInput").ap()
    g_d = nc.dram_tensor("gamma", [D], F32, kind="ExternalInput").ap()
    bt_d = nc.dram_tensor("beta", [D], F32, kind="ExternalInput").ap()
    y_d = nc.dram_tensor("y", [ROWS, D], F32, kind="ExternalOutput").ap()

    with tile.TileContext(nc) as tc:
        with tc.tile_pool(name="sb", bufs=1) as sb, \
             tc.tile_pool(name="yt_sb", bufs=3) as yt_sb, \
             tc.tile_pool(name="st_sb", bufs=4) as st_sb, \
             tc.tile_pool(name="ps", bufs=4, space="PSUM") as ps_pool:
            atq = sb.tile([128, 8, ROWS], FP16, tag="atq")
            nc.sync.dma_start(out=atq, in_=atq_d.rearrange("(hc p) i -> p hc i", p=128))
            rdn = sb.tile([128, 8, ROWS], FP16, tag="rdn")
            nc.sync.dma_start(out=rdn, in_=rdn_d.rearrange("(hc p) i -> p hc i", p=128))
            atn = sb.tile([128, 8, ROWS], FP16, tag="atn")
            nc.vector.tensor_tensor(out=atn, in0=atq, in1=rdn, op=OP.mult)
            wo = sb.tile([128, 8, D], FP16, tag="wo")
            nc.sync.dma_start(out=wo, in_=wo_d.rearrange("(hc p) m -> p hc m", p=128))
            x_sb = sb.tile([128, 4, D], F32, tag="x")
            nc.sync.dma_start(out=x_sb, in_=xr_d.rearrange("(ic p) m -> p ic m", p=128))
            gb = sb.tile([128, D], F32, tag="gb")
            nc.sync.dma_start(
                out=gb,
                in_=bass.AP(tensor=g_d.tensor, offset=g_d.offset, ap=[[0, 128]] + g_d.ap),
            )
            bb = sb.tile([128, D], F32, tag="bb")
            nc.sync.dma_start(
                out=bb,
                in_=bass.AP(tensor=bt_d.tensor, offset=bt_d.offset, ap=[[0, 128]] + bt_d.ap),
            )
            eps_t = sb.tile([128, 1], F32, tag="eps")
            nc.vector.memset(eps_t, LN_EPS)

            for ic in range(4):
                yt = yt_sb.tile([128, D], F32, tag="yt")
                for mh in range(2):
                    o_ps = ps_pool.tile([128, 512], F32, tag="o")
                    for hc in range(8):
                        nc.tensor.matmul(
                            o_ps,
                            atn[:, hc, ic * 128:(ic + 1) * 128],
                            wo[:, hc, mh * 512:(mh + 1) * 512],
                            start=(hc == 0), stop=(hc == 7),
                        )
                    nc.vector.tensor_tensor(
                        out=yt[:, mh * 512:(mh + 1) * 512],
                        in0=o_ps,
                        in1=x_sb[:, ic, mh * 512:(mh + 1) * 512],
                        op=OP.add,
                    )
                stats = st_sb.tile([128, 2, 6], F32, tag="stats")
                for sg in range(2):
                    nc.vector.bn_stats(
                        out=stats[:, sg, :], in_=yt[:, sg * 512:(sg + 1) * 512]
                    )
                mv = st_sb.tile([128, 2], F32, tag="mv")
                nc.vector.bn_aggr(out=mv, in_=stats)
                rstd = st_sb.tile([128, 1], F32, tag="rstd")
                nc.scalar.activation(
                    out=rstd, in_=mv[:, 1:2], func=AF.Sqrt, bias=eps_t, scale=1.0
                )
                nc.vector.reciprocal(out=rstd, in_=rstd)
                nc.vector.tensor_scalar(
                    out=yt, in0=yt, scalar1=mv[:, 0:1], scalar2=rstd,
                    op0=OP.subtract, op1=OP.mult,
                )
                nc.vector.tensor_tensor(out=yt, in0=yt, in1=gb, op=OP.mult)
                nc.vector.tensor_tensor(out=yt, in0=yt, in1=bb, op=OP.add)
                nc.sync.dma_start(out=y_d[ic * 128:(ic + 1) * 128, :], in_=yt)
    nc.compile()
    return nc


def _prep_a(x, w_q, w_k, w_v):
    xt = np.ascontiguousarray(x.transpose(0, 2, 1)).astype(FP16_NP)  # [B, D, L]

    def w_slice(w, c):
        return np.ascontiguousarray(
            w[2 * c:2 * c + 2].transpose(1, 0, 2).reshape(D, 2 * DQ)
        ).astype(FP16_NP)

    return [
        {
            "xt": xt,
            "wq": w_slice(w_q, c),
            "wk": w_slice(w_k, c),
            "wv": w_slice(w_v, c),
        }
        for c in range(N_CORES)
    ]


def _prep_b(res_a_results, x, w_o, ln_gamma, ln_beta):
    at_full = np.concatenate(
        [res_a_results[c]["at"][:, :128, :] for c in range(N_CORES)], axis=1
    )  # [B, H*DQ, L]
    den = np.stack(
        [res_a_results[c]["at"][:, 128:130, :] for c in range(N_CORES)], axis=1
    ).reshape(B, H, L)
    rdn_full = np.repeat((1.0 / den).astype(np.float32), DQ, axis=1)  # [B, H*DQ, L]

    ROWS = B * L // N_CORES
    wo_flat = np.ascontiguousarray(w_o.reshape(H * DQ, D)).astype(FP16_NP)
    # exact power-of-two rescale keeps both factors in fp16 range;
    # it cancels exactly in the on-device product
    at_bf = (at_full * (1.0 / 64.0)).astype(FP16_NP)
    rdn_bf = (rdn_full * 64.0).astype(FP16_NP)
    in_maps_b = []
    for c in range(N_CORES):
        b = c // (N_CORES // B)
        l0 = (c % (N_CORES // B)) * ROWS
        in_maps_b.append(
            {
                "atq": np.ascontiguousarray(at_bf[b][:, l0:l0 + ROWS]),
                "rdn": np.ascontiguousarray(rdn_bf[b][:, l0:l0 + ROWS]),
                "xr": np.ascontiguousarray(x[b, l0:l0 + ROWS]),
                "wo": wo_flat,
                "gamma": ln_gamma,
                "beta": ln_beta,
            }
        )
    return in_maps_b


def kernel(x, w_q, w_k, w_v, w_o, ln_gamma, ln_beta):
    x = np.asarray(x, dtype=np.float32)
    w_q = np.asarray(w_q, dtype=np.float32)
    w_k = np.asarray(w_k, dtype=np.float32)
    w_v = np.asarray(w_v, dtype=np.float32)
    w_o = np.asarray(w_o, dtype=np.float32)
    ln_gamma = np.asarray(ln_gamma, dtype=np.float32)
    ln_beta = np.asarray(ln_beta, dtype=np.float32)

    if "a" not in _cache:
        _cache["a"] = _build_phase_a()
    if "b" not in _cache:
        _cache["b"] = _build_phase_b()

    in_maps_a = _prep_a(x, w_q, w_k, w_v)
    res_a = run_bass_kernel_spmd(
        _cache["a"], in_maps_a, core_ids=list(range(N_CORES)), trace=False
    )
    in_maps_b = _prep_b(res_a.results, x, w_o, ln_gamma, ln_beta)
    res_b = run_bass_kernel_spmd(
        _cache["b"], in_maps_b, core_ids=list(range(N_CORES)), trace=False
    )
    y = np.concatenate([res_b.results[c]["y"] for c in range(N_CORES)], axis=0)
    return y.reshape(B, L, D)


# revision 16
# speedup vs baseline: 1.4593x; 1.0169x over previous
"""Multi-head attention block (QKV proj + softmax attention + out proj +
residual + LayerNorm) on 8 Trainium2 NeuronCores.

Sharding:
  Phase A: head-parallel — core c computes heads (2c, 2c+1) for both batch
           elements: Q/K/V projections, scores (transposed layout), exp,
           unnormalized P@V and softmax denominators.
  Phase B: row-parallel — core c computes 512 rows of the flattened (B*L)
           output: per-head normalization, output projection (contracting
           over all 16 heads), residual add and LayerNorm.

Matmul operands in fp16 (PE streams 2-byte operands at full rate); all
accumulation in fp32 PSUM; softmax/LayerNorm arithmetic in fp32.
exp uses a fixed -2.0 bias to keep fp16 P values in range; it cancels in
the softmax normalization since denominators use the same biased values.
"""

import sys

if "/opt/trn_rl_repo" not in sys.path:
    sys.path.insert(0, "/opt/trn_rl_repo")

import ml_dtypes
import numpy as np

import concourse.bass as bass
import concourse.tile as tile
from concourse import bacc, mybir
from concourse.bass_utils import run_bass_kernel_spmd
from concourse.masks import make_identity

B, L, D, H, DQ = 2, 2048, 1024, 16, 64
N_CORES = 8
LN_EPS = 1e-5
F32 = mybir.dt.float32
FP16 = mybir.dt.float16
AF = mybir.ActivationFunctionType
OP = mybir.AluOpType
FP16_NP = np.float16

_cache = {}


def _build_phase_a():
    nc = bacc.Bacc("TRN2", target_bir_lowering=False, debug=False, num_devices=N_CORES)
    xt_d = nc.dram_tensor("xt", [B, D, L], FP16, kind="ExternalInput").ap()
    wq_d = nc.dram_tensor("wq", [D, 128], FP16, kind="ExternalInput").ap()
    wk_d = nc.dram_tensor("wk", [D, 128], FP16, kind="ExternalInput").ap()
    wv_d = nc.dram_tensor("wv", [D, 128], FP16, kind="ExternalInput").ap()
    # rows 0..127: A^T (2 heads x 64), rows 128..129: softmax denominators
    at_d = nc.dram_tensor("at", [B, 130, L], F32, kind="ExternalOutput").ap()

    with tile.TileContext(nc) as tc:
        with tc.tile_pool(name="singles", bufs=1) as singles, \
             tc.tile_pool(name="xt_pool", bufs=2) as xt_pool, \
             tc.tile_pool(name="proj_sb", bufs=2) as proj_sb, \
             tc.tile_pool(name="pt_sb", bufs=6) as pt_sb, \
             tc.tile_pool(name="out_sb", bufs=3) as out_sb, \
             tc.tile_pool(name="ps_mm", bufs=3, space="PSUM") as ps_mm, \
             tc.tile_pool(name="ps_apv", bufs=2, space="PSUM") as ps_apv:
            w_sb = {}
            for nm, d in (("wq", wq_d), ("wk", wk_d), ("wv", wv_d)):
                t = singles.tile([128, 8, 128], FP16, tag=nm)
                nc.sync.dma_start(out=t, in_=d.rearrange("(mc p) h -> p mc h", p=128))
                w_sb[nm] = t
            ident_f = singles.tile([128, 128], F32, tag="ident_f")
            make_identity(nc, ident_f)
            ident = singles.tile([128, 128], FP16, tag="ident")
            nc.vector.tensor_copy(out=ident, in_=ident_f)
            exp_bias = singles.tile([128, 1], F32, tag="exp_bias")
            nc.vector.memset(exp_bias, -8.0)

            xt, qt, kt, vt, vsb = {}, {}, {}, {}, {}
            for b in range(B):
                xt[b] = xt_pool.tile([128, 8, L], FP16, tag="xt", name=f"xt{b}")
                for mc in range(8):
                    nc.sync.dma_start(
                        out=xt[b][:, mc, :],
                        in_=xt_d[b].rearrange("(mc p) l -> p mc l", p=128)[:, mc, :],
                    )
                qt[b] = proj_sb.tile([128, L], FP16, tag="qt", name=f"qt{b}")
                kt[b] = proj_sb.tile([128, L], FP16, tag="kt", name=f"kt{b}")
                vt[b] = proj_sb.tile([128, L], FP16, tag="vt", name=f"vt{b}")
                vsb[b] = proj_sb.tile([128, 2, 16, 128], FP16, tag="vsb", name=f"vsb{b}")

            def proj_group(b, dst, w, it):
                sl = slice(it * 512, (it + 1) * 512)
                ps = ps_mm.tile([128, 512], F32, tag="mm", name="ps")
                for mc in range(8):
                    nc.tensor.matmul(
                        ps, w[:, mc, :], xt[b][:, mc, sl],
                        start=(mc == 0), stop=(mc == 7),
                    )
                nc.vector.tensor_copy(out=dst[:, sl], in_=ps)

            def v_init(b):
                nc.vector.memset(vsb[b], 0.0)
                nc.vector.tensor_copy(
                    out=vsb[b][:, :, :, 64:65], in_=ones_col[:, :, :, :]
                )

            def v_transpose(b, h, jc):
                ps = ps_mm.tile([128, 512], FP16, tag="mm", name="ps")
                nc.tensor.transpose(
                    ps[:, 0:64],
                    vt[b][h * 64:(h + 1) * 64, jc * 128:(jc + 1) * 128],
                    ident[h * 64:(h + 1) * 64, h * 64:(h + 1) * 64],
                )
                nc.vector.tensor_copy(out=vsb[b][:, h, jc, 0:64], in_=ps[:, 0:64])

            ones_col = singles.tile([128, 2, 16, 1], FP16, tag="ones")
            nc.vector.memset(ones_col, 1.0)

            # b=0 projections: Q,K first so the softmax exp stream starts
            # as early as possible, then V (+transposes to natural layout)
            for it in range(4):
                proj_group(0, qt[0], w_sb["wq"], it)
            for it in range(4):
                proj_group(0, kt[0], w_sb["wk"], it)
            for it in range(4):
                proj_group(0, vt[0], w_sb["wv"], it)
            v_init(0)
            for jc in range(16):
                for h in range(2):
                    v_transpose(0, h, jc)

            # deferred b=1 projection work, interleaved into b=0's
            # attention loop to fill PE stalls while ACT (exp) paces it
            filler = []
            for it in range(4):
                filler.append(lambda it=it: proj_group(1, qt[1], w_sb["wq"], it))
            for it in range(4):
                filler.append(lambda it=it: proj_group(1, kt[1], w_sb["wk"], it))
            for it in range(4):
                filler.append(lambda it=it: proj_group(1, vt[1], w_sb["wv"], it))
            filler.append(lambda: v_init(1))
            for jc in range(16):
                filler.append(lambda jc=jc: (v_transpose(1, 0, jc), v_transpose(1, 1, jc)))

            def attention(b, emit_filler):
                for it in range(4):
                    i_sl = slice(it * 512, (it + 1) * 512)
                    apv = [
                        ps_apv.tile([128, 512], F32, tag="apv", name=f"apv{_h}")
                        for _h in range(2)
                    ]
                    for jc in range(16):
                        # one ST tile holds both heads' scores for this jc —
                        # the shared exp forces the two K=64 matmuls adjacent
                        # so they run concurrently in disjoint PE row groups
                        st = ps_mm.tile([128, 1024], F32, tag="mm", name="st")
                        for h in range(2):
                            hs = slice(h * 64, (h + 1) * 64)
                            nc.tensor.matmul(
                                st[:, h * 512:(h + 1) * 512],
                                kt[b][hs, jc * 128:(jc + 1) * 128],
                                qt[b][hs, i_sl],
                                start=True, stop=True,
                            )
                        ptt = pt_sb.tile([128, 1024], FP16, tag="pt", name="pt")
                        nc.scalar.activation(
                            out=ptt, in_=st, func=AF.Exp,
                            scale=1.0 / (DQ ** 0.5), bias=exp_bias,
                        )
                        for h in range(2):
                            nc.tensor.matmul(
                                apv[h],
                                vsb[b][:, h, jc, :],
                                ptt[:, h * 512:(h + 1) * 512],
                                start=(jc == 0),
                                stop=(jc == 15),
                            )
                        if emit_filler and filler:
                            filler.pop(0)()
                    for h in range(2):
                        o_sb = out_sb.tile([65, 512], F32, tag="o")
                        nc.vector.tensor_copy(out=o_sb, in_=apv[h][0:65, :])
                        nc.sync.dma_start(
                            out=at_d[b, h * 64:(h + 1) * 64, i_sl], in_=o_sb[0:64, :]
                        )
                        nc.sync.dma_start(
                            out=at_d[b, 128 + h:129 + h, i_sl], in_=o_sb[64:65, :]
                        )

            attention(0, True)
            while filler:
                filler.pop(0)()
            attention(1, False)
    nc.compile()
    return nc


def _build_phase_b():
    nc = bacc.Bacc("TRN2", target_bir_lowering=False, debug=False, num_devices=N_CORES)
    ROWS = B * L // N_CORES  # 512
    atq_d = nc.dram_tensor("atq", [H * DQ, ROWS], FP16, kind="ExternalInput").ap()
    rdn_d = nc.dram_tensor("rdn", [H * DQ, ROWS], FP16, kind="ExternalInput").ap()
    xr_d = nc.dram_tensor("xr", [ROWS, D], F32, kind="ExternalInput").ap()
    wo_d = nc.dram_tensor("wo", [H * DQ, D], FP16, kind="ExternalInput").ap()
    g_d = nc.dram_tensor("gamma", [D], F32, kind="ExternalInput").ap()
    bt_d = nc.dram_tensor("beta", [D], F32, kind="ExternalInput").ap()
    y_d = nc.dram_tensor("y", [ROWS, D], F32, kind="ExternalOutput").ap()

    with tile.TileContext(nc) as tc:
        with tc.tile_pool(name="sb", bufs=1) as sb, \
             tc.tile_pool(name="yt_sb", bufs=3) as yt_sb, \
             tc.tile_pool(name="st_sb", bufs=4) as st_sb, \
             tc.tile_pool(name="ps", bufs=4, space="PSUM") as ps_pool:
            atq = sb.tile([128, 8, ROWS], FP16, tag="atq")
            nc.sync.dma_start(out=atq, in_=atq_d.rearrange("(hc p) i -> p hc i", p=128))
            rdn = sb.tile([128, 8, ROWS], FP16, tag="rdn")
            nc.sync.dma_start(out=rdn, in_=rdn_d.rearrange("(hc p) i -> p hc i", p=128))
            atn = sb.tile([128, 8, ROWS], FP16, tag="atn")
            nc.vector.tensor_tensor(out=atn, in0=atq, in1=rdn, op=OP.mult)
            wo = sb.tile([128, 8, D], FP16, tag="wo")
            nc.sync.dma_start(out=wo, in_=wo_d.rearrange("(hc p) m -> p hc m", p=128))
            x_sb = sb.tile([128, 4, D], F32, tag="x")
            nc.sync.dma_start(out=x_sb, in_=xr_d.rearrange("(ic p) m -> p ic m", p=128))
            gb = sb.tile([128, D], F32, tag="gb")
            nc.sync.dma_start(
                out=gb,
                in_=bass.AP(tensor=g_d.tensor, offset=g_d.offset, ap=[[0, 128]] + g_d.ap),
            )
            bb = sb.tile([128, D], F32, tag="bb")
            nc.sync.dma_start(
                out=bb,
                in_=bass.AP(tensor=bt_d.tensor, offset=bt_d.offset, ap=[[0, 128]] + bt_d.ap),
            )
            eps_t = sb.tile([128, 1], F32, tag="eps")
            nc.vector.memset(eps_t, LN_EPS)

            for ic in range(4):
                yt = yt_sb.tile([128, D], F32, tag="yt")
                for mh in range(2):
                    o_ps = ps_pool.tile([128, 512], F32, tag="o")
                    for hc in range(8):
                        nc.tensor.matmul(
                            o_ps,
                            atn[:, hc, ic * 128:(ic + 1) * 128],
                            wo[:, hc, mh * 512:(mh + 1) * 512],
                            start=(hc == 0), stop=(hc == 7),
                        )
                    nc.vector.tensor_tensor(
                        out=yt[:, mh * 512:(mh + 1) * 512],
                        in0=o_ps,
                        in1=x_sb[:, ic, mh * 512:(mh + 1) * 512],
                        op=OP.add,
                    )
                stats = st_sb.tile([128, 2, 6], F32, tag="stats")
                for sg in range(2):
                    nc.vector.bn_stats(
                        out=stats[:, sg, :], in_=yt[:, sg * 512:(sg + 1) * 512]
                    )
                mv = st_sb.tile([128, 2], F32, tag="mv")
                nc.vector.bn_aggr(out=mv, in_=stats)
                rstd = st_sb.tile([128, 1], F32, tag="rstd")
                nc.scalar.activation(
                    out=rstd, in_=mv[:, 1:2], func=AF.Sqrt, bias=eps_t, scale=1.0
                )
                nc.vector.reciprocal(out=rstd, in_=rstd)
                nc.vector.tensor_scalar(
                    out=yt, in0=yt, scalar1=mv[:, 0:1], scalar2=rstd,
                    op0=OP.subtract, op1=OP.mult,
                )
                nc.vector.tensor_tensor(out=yt, in0=yt, in1=gb, op=OP.mult)
                nc.vector.tensor_tensor(out=yt, in0=yt, in1=bb, op=OP.add)
                nc.sync.dma_start(out=y_d[ic * 128:(ic + 1) * 128, :], in_=yt)
    nc.compile()
    return nc


def _prep_a(x, w_q, w_k, w_v):
    xt = np.ascontiguousarray(x.transpose(0, 2, 1)).astype(FP16_NP)  # [B, D, L]

    def w_slice(w, c):
        return np.ascontiguousarray(
            w[2 * c:2 * c + 2].transpose(1, 0, 2).reshape(D, 2 * DQ)
        ).astype(FP16_NP)

    return [
        {
            "xt": xt,
            "wq": w_slice(w_q, c),
            "wk": w_slice(w_k, c),
            "wv": w_slice(w_v, c),
        }
        for c in range(N_CORES)
    ]


def _prep_b(res_a_results, x, w_o, ln_gamma, ln_beta):
    at_full = np.concatenate(
        [res_a_results[c]["at"][:, :128, :] for c in range(N_CORES)], axis=1
    )  # [B, H*DQ, L]
    den = np.stack(
        [res_a_results[c]["at"][:, 128:130, :] for c in range(N_CORES)], axis=1
    ).reshape(B, H, L)
    rdn_full = np.repeat((1.0 / den).astype(np.float32), DQ, axis=1)  # [B, H*DQ, L]

    ROWS = B * L // N_CORES
    wo_flat = np.ascontiguousarray(w_o.reshape(H * DQ, D)).astype(FP16_NP)
    # exact power-of-two rescale keeps both factors in fp16 range;
    # it cancels exactly in the on-device product
    at_bf = (at_full * (1.0 / 64.0)).astype(FP16_NP)
    rdn_bf = (rdn_full * 64.0).astype(FP16_NP)
    in_maps_b = []
    for c in range(N_CORES):
        b = c // (N_CORES // B)
        l0 = (c % (N_CORES // B)) * ROWS
        in_maps_b.append(
            {
                "atq": np.ascontiguousarray(at_bf[b][:, l0:l0 + ROWS]),
                "rdn": np.ascontiguousarray(rdn_bf[b][:, l0:l0 + ROWS]),
                "xr": np.ascontiguousarray(x[b, l0:l0 + ROWS]),
                "wo": wo_flat,
                "gamma": ln_gamma,
                "beta": ln_beta,
            }
        )
    return in_maps_b


def kernel(x, w_q, w_k, w_v, w_o, ln_gamma, ln_beta):
    x = np.asarray(x, dtype=np.float32)
    w_q = np.asarray(w_q, dtype=np.float32)
    w_k = np.asarray(w_k, dtype=np.float32)
    w_v = np.asarray(w_v, dtype=np.float32)
    w_o = np.asarray(w_o, dtype=np.float32)
    ln_gamma = np.asarray(ln_gamma, dtype=np.float32)
    ln_beta = np.asarray(ln_beta, dtype=np.float32)

    if "a" not in _cache:
        _cache["a"] = _build_phase_a()
    if "b" not in _cache:
        _cache["b"] = _build_phase_b()

    in_maps_a = _prep_a(x, w_q, w_k, w_v)
    res_a = run_bass_kernel_spmd(
        _cache["a"], in_maps_a, core_ids=list(range(N_CORES)), trace=False
    )
    in_maps_b = _prep_b(res_a.results, x, w_o, ln_gamma, ln_beta)
    res_b = run_bass_kernel_spmd(
        _cache["b"], in_maps_b, core_ids=list(range(N_CORES)), trace=False
    )
    y = np.concatenate([res_b.results[c]["y"] for c in range(N_CORES)], axis=0)
    return y.reshape(B, L, D)


# revision 18
# speedup vs baseline: 1.4768x; 1.0120x over previous
"""Multi-head attention block (QKV proj + softmax attention + out proj +
residual + LayerNorm) on 8 Trainium2 NeuronCores.

Sharding:
  Phase A: head-parallel — core c computes heads (2c, 2c+1) for both batch
           elements: Q/K/V projections, scores (transposed layout), exp,
           unnormalized P@V and softmax denominators.
  Phase B: row-parallel — core c computes 512 rows of the flattened (B*L)
           output: per-head normalization, output projection (contracting
           over all 16 heads), residual add and LayerNorm.

Matmul operands in fp16 (PE streams 2-byte operands at full rate); all
accumulation in fp32 PSUM; softmax/LayerNorm arithmetic in fp32.
exp uses a fixed -2.0 bias to keep fp16 P values in range; it cancels in
the softmax normalization since denominators use the same biased values.
"""

import sys

if "/opt/trn_rl_repo" not in sys.path:
    sys.path.insert(0, "/opt/trn_rl_repo")

import ml_dtypes
import numpy as np

import concourse.bass as bass
import concourse.tile as tile
from concourse import bacc, mybir
from concourse.bass_utils import run_bass_kernel_spmd
from concourse.masks import make_identity

B, L, D, H, DQ = 2, 2048, 1024, 16, 64
N_CORES = 8
LN_EPS = 1e-5
F32 = mybir.dt.float32
FP16 = mybir.dt.float16
AF = mybir.ActivationFunctionType
OP = mybir.AluOpType
FP16_NP = np.float16

_cache = {}


def _build_phase_a():
    nc = bacc.Bacc("TRN2", target_bir_lowering=False, debug=False, num_devices=N_CORES)
    xt_d = nc.dram_tensor("xt", [B, D, L], FP16, kind="ExternalInput").ap()
    wq_d = nc.dram_tensor("wq", [D, 128], FP16, kind="ExternalInput").ap()
    wk_d = nc.dram_tensor("wk", [D, 128], FP16, kind="ExternalInput").ap()
    wv_d = nc.dram_tensor("wv", [D, 128], FP16, kind="ExternalInput").ap()
    # rows 0..127: A^T (2 heads x 64), rows 128..129: softmax denominators
    at_d = nc.dram_tensor("at", [B, 130, L], F32, kind="ExternalOutput").ap()

    with tile.TileContext(nc) as tc:
        with tc.tile_pool(name="singles", bufs=1) as singles, \
             tc.tile_pool(name="xt_pool", bufs=2) as xt_pool, \
             tc.tile_pool(name="proj_sb", bufs=2) as proj_sb, \
             tc.tile_pool(name="pt_sb", bufs=6) as pt_sb, \
             tc.tile_pool(name="out_sb", bufs=3) as out_sb, \
             tc.tile_pool(name="ps_mm", bufs=3, space="PSUM") as ps_mm, \
             tc.tile_pool(name="ps_apv", bufs=2, space="PSUM") as ps_apv:
            w_sb = {}
            for nm, d in (("wq", wq_d), ("wk", wk_d), ("wv", wv_d)):
                t = singles.tile([128, 8, 128], FP16, tag=nm)
                nc.sync.dma_start(out=t, in_=d.rearrange("(mc p) h -> p mc h", p=128))
                w_sb[nm] = t
            ident_f = singles.tile([128, 128], F32, tag="ident_f")
            make_identity(nc, ident_f)
            ident = singles.tile([128, 128], FP16, tag="ident")
            nc.vector.tensor_copy(out=ident, in_=ident_f)
            exp_bias = singles.tile([128, 1], F32, tag="exp_bias")
            nc.vector.memset(exp_bias, -8.0)

            xt, qt, kt, vt, vsb = {}, {}, {}, {}, {}
            for b in range(B):
                xt[b] = xt_pool.tile([128, 8, L], FP16, tag="xt", name=f"xt{b}")
                for mc in range(8):
                    nc.sync.dma_start(
                        out=xt[b][:, mc, :],
                        in_=xt_d[b].rearrange("(mc p) l -> p mc l", p=128)[:, mc, :],
                    )
                qt[b] = proj_sb.tile([128, L], FP16, tag="qt", name=f"qt{b}")
                kt[b] = proj_sb.tile([128, L], FP16, tag="kt", name=f"kt{b}")
                vt[b] = proj_sb.tile([128, L], FP16, tag="vt", name=f"vt{b}")
                vsb[b] = proj_sb.tile([128, 2, 16, 128], FP16, tag="vsb", name=f"vsb{b}")

            def proj_group(b, dst, w, it):
                sl = slice(it * 512, (it + 1) * 512)
                ps = ps_mm.tile([128, 512], F32, tag="mm", name="ps")
                for mc in range(8):
                    nc.tensor.matmul(
                        ps, w[:, mc, :], xt[b][:, mc, sl],
                        start=(mc == 0), stop=(mc == 7),
                    )
                nc.vector.tensor_copy(out=dst[:, sl], in_=ps)

            def v_init(b):
                nc.vector.memset(vsb[b], 0.0)
                nc.vector.tensor_copy(
                    out=vsb[b][:, :, :, 64:65], in_=ones_col[:, :, :, :]
                )

            def v_transpose(b, h, jc):
                ps = ps_mm.tile([128, 512], FP16, tag="mm", name="ps")
                nc.tensor.transpose(
                    ps[:, 0:64],
                    vt[b][h * 64:(h + 1) * 64, jc * 128:(jc + 1) * 128],
                    ident[h * 64:(h + 1) * 64, h * 64:(h + 1) * 64],
                )
                nc.vector.tensor_copy(out=vsb[b][:, h, jc, 0:64], in_=ps[:, 0:64])

            ones_col = singles.tile([128, 2, 16, 1], FP16, tag="ones")
            nc.vector.memset(ones_col, 1.0)

            # b=0 projections: Q,K first so the softmax exp stream starts
            # as early as possible, then V (+transposes to natural layout)
            # b=0: K fully (scores need all of KT), Q for the first
            # i-block, then V + transposes (PV needs them); Q(it1-3) and
            # all of b=1 are deferred into the attention loop as fillers
            for it in range(4):
                proj_group(0, kt[0], w_sb["wk"], it)
            proj_group(0, qt[0], w_sb["wq"], 0)
            for it in range(4):
                proj_group(0, vt[0], w_sb["wv"], it)
            v_init(0)
            for jc in range(16):
                for h in range(2):
                    v_transpose(0, h, jc)

            # deferred projection work, interleaved into b=0's attention
            # loop to fill PE stalls while ACT (exp) paces it
            filler = []
            for it in range(1, 4):
                filler.append(lambda it=it: proj_group(0, qt[0], w_sb["wq"], it))
            for it in range(4):
                filler.append(lambda it=it: proj_group(1, qt[1], w_sb["wq"], it))
            for it in range(4):
                filler.append(lambda it=it: proj_group(1, kt[1], w_sb["wk"], it))
            for it in range(4):
                filler.append(lambda it=it: proj_group(1, vt[1], w_sb["wv"], it))
            filler.append(lambda: v_init(1))
            for jc in range(16):
                filler.append(lambda jc=jc: (v_transpose(1, 0, jc), v_transpose(1, 1, jc)))

            def attention(b, emit_filler):
                for it in range(4):
                    i_sl = slice(it * 512, (it + 1) * 512)
                    apv = [
                        ps_apv.tile([128, 512], F32, tag="apv", name=f"apv{_h}")
                        for _h in range(2)
                    ]
                    def score_pair(jc):
                        # one ST tile holds both heads' scores for this jc —
                        # the shared exp forces the two K=64 matmuls adjacent
                        # so they run concurrently in disjoint PE row groups
                        st = ps_mm.tile([128, 1024], F32, tag="mm", name="st")
                        for h in range(2):
                            hs = slice(h * 64, (h + 1) * 64)
                            nc.tensor.matmul(
                                st[:, h * 512:(h + 1) * 512],
                                kt[b][hs, jc * 128:(jc + 1) * 128],
                                qt[b][hs, i_sl],
                                start=True, stop=True,
                            )
                        return st

                    st_cur = score_pair(0)
                    for jc in range(16):
                        ptt = pt_sb.tile([128, 1024], FP16, tag="pt", name="pt")
                        nc.scalar.activation(
                            out=ptt, in_=st_cur, func=AF.Exp,
                            scale=1.0 / (DQ ** 0.5), bias=exp_bias,
                        )
                        if jc + 1 < 16:
                            st_cur = score_pair(jc + 1)
                        for h in range(2):
                            nc.tensor.matmul(
                                apv[h],
                                vsb[b][:, h, jc, :],
                                ptt[:, h * 512:(h + 1) * 512],
                                start=(jc == 0),
                                stop=(jc == 15),
                            )
                        if emit_filler and filler:
                            filler.pop(0)()
                    for h in range(2):
                        o_sb = out_sb.tile([65, 512], F32, tag="o")
                        nc.vector.tensor_copy(out=o_sb, in_=apv[h][0:65, :])
                        nc.sync.dma_start(
                            out=at_d[b, h * 64:(h + 1) * 64, i_sl], in_=o_sb[0:64, :]
                        )
                        nc.sync.dma_start(
                            out=at_d[b, 128 + h:129 + h, i_sl], in_=o_sb[64:65, :]
                        )

            attention(0, True)
            while filler:
                filler.pop(0)()
            attention(1, False)
    nc.compile()
    return nc


def _build_phase_b():
    nc = bacc.Bacc("TRN2", target_bir_lowering=False, debug=False, num_devices=N_CORES)
    ROWS = B * L // N_CORES  # 512
    atq_d = nc.dram_tensor("atq", [H * DQ, ROWS], FP16, kind="ExternalInput").ap()
    rdn_d = nc.dram_tensor("rdn", [H * DQ, ROWS], FP16, kind="ExternalInput").ap()
    xr_d = nc.dram_tensor("xr", [ROWS, D], F32, kind="ExternalInput").ap()
    wo_d = nc.dram_tensor("wo", [H * DQ, D], FP16, kind="ExternalInput").ap()
    g_d = nc.dram_tensor("gamma", [D], F32, kind="ExternalInput").ap()
    bt_d = nc.dram_tensor("beta", [D], F32, kind="ExternalInput").ap()
    y_d = nc.dram_tensor("y", [ROWS, D], F32, kind="ExternalOutput").ap()

    with tile.TileContext(nc) as tc:
        with tc.tile_pool(name="sb", bufs=1) as sb, \
             tc.tile_pool(name="yt_sb", bufs=3) as yt_sb, \
             tc.tile_pool(name="st_sb", bufs=4) as st_sb, \
             tc.tile_pool(name="ps", bufs=4, space="PSUM") as ps_pool:
            atq = sb.tile([128, 8, ROWS], FP16, tag="atq")
            nc.sync.dma_start(out=atq, in_=atq_d.rearrange("(hc p) i -> p hc i", p=128))
            rdn = sb.tile([128, 8, ROWS], FP16, tag="rdn")
            nc.sync.dma_start(out=rdn, in_=rdn_d.rearrange("(hc p) i -> p hc i", p=128))
            atn = sb.tile([128, 8, ROWS], FP16, tag="atn")
            nc.vector.tensor_tensor(out=atn, in0=atq, in1=rdn, op=OP.mult)
            wo = sb.tile([128, 8, D], FP16, tag="wo")
            nc.sync.dma_start(out=wo, in_=wo_d.rearrange("(hc p) m -> p hc m", p=128))
            x_sb = sb.tile([128, 4, D], F32, tag="x")
            nc.sync.dma_start(out=x_sb, in_=xr_d.rearrange("(ic p) m -> p ic m", p=128))
            gb = sb.tile([128, D], F32, tag="gb")
            nc.sync.dma_start(
                out=gb,
                in_=bass.AP(tensor=g_d.tensor, offset=g_d.offset, ap=[[0, 128]] + g_d.ap),
            )
            bb = sb.tile([128, D], F32, tag="bb")
            nc.sync.dma_start(
                out=bb,
                in_=bass.AP(tensor=bt_d.tensor, offset=bt_d.offset, ap=[[0, 128]] + bt_d.ap),
            )
            eps_t = sb.tile([128, 1], F32, tag="eps")
            nc.vector.memset(eps_t, LN_EPS)

            for ic in range(4):
                yt = yt_sb.tile([128, D], F32, tag="yt")
                for mh in range(2):
                    o_ps = ps_pool.tile([128, 512], F32, tag="o")
                    for hc in range(8):
                        nc.tensor.matmul(
                            o_ps,
                            atn[:, hc, ic * 128:(ic + 1) * 128],
                            wo[:, hc, mh * 512:(mh + 1) * 512],
                            start=(hc == 0), stop=(hc == 7),
                        )
                    nc.vector.tensor_tensor(
                        out=yt[:, mh * 512:(mh + 1) * 512],
                        in0=o_ps,
                        in1=x_sb[:, ic, mh * 512:(mh + 1) * 512],
                        op=OP.add,
                    )
                stats = st_sb.tile([128, 2, 6], F32, tag="stats")
                for sg in range(2):
                    nc.vector.bn_stats(
                        out=stats[:, sg, :], in_=yt[:, sg * 512:(sg + 1) * 512]
                    )
                mv = st_sb.tile([128, 2], F32, tag="mv")
                nc.vector.bn_aggr(out=mv, in_=stats)
                rstd = st_sb.tile([128, 1], F32, tag="rstd")
                nc.scalar.activation(
                    out=rstd, in_=mv[:, 1:2], func=AF.Sqrt, bias=eps_t, scale=1.0
                )
                nc.vector.reciprocal(out=rstd, in_=rstd)
                nc.vector.tensor_scalar(
                    out=yt, in0=yt, scalar1=mv[:, 0:1], scalar2=rstd,
                    op0=OP.subtract, op1=OP.mult,
                )
                nc.vector.tensor_tensor(out=yt, in0=yt, in1=gb, op=OP.mult)
                nc.vector.tensor_tensor(out=yt, in0=yt, in1=bb, op=OP.add)
                nc.sync.dma_start(out=y_d[ic * 128:(ic + 1) * 128, :], in_=yt)
    nc.compile()
    return nc


def _prep_a(x, w_q, w_k, w_v):
    xt = np.ascontiguousarray(x.transpose(0, 2, 1)).astype(FP16_NP)  # [B, D, L]

    def w_slice(w, c):
        return np.ascontiguousarray(
            w[2 * c:2 * c + 2].transpose(1, 0, 2).reshape(D, 2 * DQ)
        ).astype(FP16_NP)

    return [
        {
            "xt": xt,
            "wq": w_slice(w_q, c),
            "wk": w_slice(w_k, c),
            "wv": w_slice(w_v, c),
        }
        for c in range(N_CORES)
    ]


def _prep_b(res_a_results, x, w_o, ln_gamma, ln_beta):
    at_full = np.concatenate(
        [res_a_results[c]["at"][:, :128, :] for c in range(N_CORES)], axis=1
    )  # [B, H*DQ, L]
    den = np.stack(
        [res_a_results[c]["at"][:, 128:130, :] for c in range(N_CORES)], axis=1
    ).reshape(B, H, L)
    rdn_full = np.repeat((1.0 / den).astype(np.float32), DQ, axis=1)  # [B, H*DQ, L]

    ROWS = B * L // N_CORES
    wo_flat = np.ascontiguousarray(w_o.reshape(H * DQ, D)).astype(FP16_NP)
    # exact power-of-two rescale keeps both factors in fp16 range;
    # it cancels exactly in the on-device product
    at_bf = (at_full * (1.0 / 64.0)).astype(FP16_NP)
    rdn_bf = (rdn_full * 64.0).astype(FP16_NP)
    in_maps_b = []
    for c in range(N_CORES):
        b = c // (N_CORES // B)
        l0 = (c % (N_CORES // B)) * ROWS
        in_maps_b.append(
            {
                "atq": np.ascontiguousarray(at_bf[b][:, l0:l0 + ROWS]),
                "rdn": np.ascontiguousarray(rdn_bf[b][:, l0:l0 + ROWS]),
                "xr": np.ascontiguousarray(x[b, l0:l0 + ROWS]),
                "wo": wo_flat,
                "gamma": ln_gamma,
                "beta": ln_beta,
            }
        )
    return in_maps_b


def kernel(x, w_q, w_k, w_v, w_o, ln_gamma, ln_beta):
    x = np.asarray(x, dtype=np.float32)
    w_q = np.asarray(w_q, dtype=np.float32)
    w_k = np.asarray(w_k, dtype=np.float32)
    w_v = np.asarray(w_v, dtype=np.float32)
    w_o = np.asarray(w_o, dtype=np.float32)
    ln_gamma = np.asarray(ln_gamma, dtype=np.float32)
    ln_beta = np.asarray(ln_beta, dtype=np.float32)

    if "a" not in _cache:
        _cache["a"] = _build_phase_a()
    if "b" not in _cache:
        _cache["b"] = _build_phase_b()

    in_maps_a = _prep_a(x, w_q, w_k, w_v)
    res_a = run_bass_kernel_spmd(
        _cache["a"], in_maps_a, core_ids=list(range(N_CORES)), trace=False
    )
    in_maps_b = _prep_b(res_a.results, x, w_o, ln_gamma, ln_beta)
    res_b = run_bass_kernel_spmd(
        _cache["b"], in_maps_b, core_ids=list(range(N_CORES)), trace=False
    )
    y = np.concatenate([res_b.results[c]["y"] for c in range(N_CORES)], axis=0)
    return y.reshape(B, L, D)


# revision 19
# speedup vs baseline: 1.4905x; 1.0093x over previous
"""Multi-head attention block (QKV proj + softmax attention + out proj +
residual + LayerNorm) on 8 Trainium2 NeuronCores.

Sharding:
  Phase A: head-parallel — core c computes heads (2c, 2c+1) for both batch
           elements: Q/K/V projections, scores (transposed layout), exp,
           unnormalized P@V and softmax denominators.
  Phase B: row-parallel — core c computes 512 rows of the flattened (B*L)
           output: per-head normalization, output projection (contracting
           over all 16 heads), residual add and LayerNorm.

Matmul operands in fp16 (PE streams 2-byte operands at full rate); all
accumulation in fp32 PSUM; softmax/LayerNorm arithmetic in fp32.
exp uses a fixed -2.0 bias to keep fp16 P values in range; it cancels in
the softmax normalization since denominators use the same biased values.
"""

import sys

if "/opt/trn_rl_repo" not in sys.path:
    sys.path.insert(0, "/opt/trn_rl_repo")

import ml_dtypes
import numpy as np

import concourse.bass as bass
import concourse.tile as tile
from concourse import bacc, mybir
from concourse.bass_utils import run_bass_kernel_spmd
from concourse.masks import make_identity

B, L, D, H, DQ = 2, 2048, 1024, 16, 64
N_CORES = 8
LN_EPS = 1e-5
F32 = mybir.dt.float32
FP16 = mybir.dt.float16
AF = mybir.ActivationFunctionType
OP = mybir.AluOpType
FP16_NP = np.float16

_cache = {}


def _build_phase_a():
    nc = bacc.Bacc("TRN2", target_bir_lowering=False, debug=False, num_devices=N_CORES)
    xt_d = nc.dram_tensor("xt", [B, D, L], FP16, kind="ExternalInput").ap()
    wq_d = nc.dram_tensor("wq", [D, 128], FP16, kind="ExternalInput").ap()
    wk_d = nc.dram_tensor("wk", [D, 128], FP16, kind="ExternalInput").ap()
    wv_d = nc.dram_tensor("wv", [D, 128], FP16, kind="ExternalInput").ap()
    # rows 0..127: A^T (2 heads x 64), rows 128..129: softmax denominators
    at_d = nc.dram_tensor("at", [B, 130, L], F32, kind="ExternalOutput").ap()

    with tile.TileContext(nc) as tc:
        with tc.tile_pool(name="singles", bufs=1) as singles, \
             tc.tile_pool(name="xt_pool", bufs=2) as xt_pool, \
             tc.tile_pool(name="proj_sb", bufs=2) as proj_sb, \
             tc.tile_pool(name="pt_sb", bufs=6) as pt_sb, \
             tc.tile_pool(name="out_sb", bufs=3) as out_sb, \
             tc.tile_pool(name="ps_mm", bufs=3, space="PSUM") as ps_mm, \
             tc.tile_pool(name="ps_apv", bufs=2, space="PSUM") as ps_apv:
            w_sb = {}
            for nm, d in (("wq", wq_d), ("wk", wk_d), ("wv", wv_d)):
                t = singles.tile([128, 8, 128], FP16, tag=nm)
                nc.sync.dma_start(out=t, in_=d.rearrange("(mc p) h -> p mc h", p=128))
                w_sb[nm] = t
            ident_f = singles.tile([128, 128], F32, tag="ident_f")
            make_identity(nc, ident_f)
            ident = singles.tile([128, 128], FP16, tag="ident")
            nc.vector.tensor_copy(out=ident, in_=ident_f)
            exp_bias = singles.tile([128, 1], F32, tag="exp_bias")
            nc.vector.memset(exp_bias, -8.0)

            xt, qt, kt, vt, vsb = {}, {}, {}, {}, {}
            for b in range(B):
                xt[b] = xt_pool.tile([128, 8, L], FP16, tag="xt", name=f"xt{b}")
                for mc in range(8):
                    nc.sync.dma_start(
                        out=xt[b][:, mc, :],
                        in_=xt_d[b].rearrange("(mc p) l -> p mc l", p=128)[:, mc, :],
                    )
                qt[b] = proj_sb.tile([128, L], FP16, tag="qt", name=f"qt{b}")
                kt[b] = proj_sb.tile([128, L], FP16, tag="kt", name=f"kt{b}")
                vt[b] = proj_sb.tile([128, L], FP16, tag="vt", name=f"vt{b}")
                vsb[b] = proj_sb.tile([128, 2, 16, 128], FP16, tag="vsb", name=f"vsb{b}")

            def proj_group(b, dst, w, it):
                sl = slice(it * 512, (it + 1) * 512)
                ps = ps_mm.tile([128, 512], F32, tag="mm", name="ps")
                for mc in range(8):
                    nc.tensor.matmul(
                        ps, w[:, mc, :], xt[b][:, mc, sl],
                        start=(mc == 0), stop=(mc == 7),
                    )
                nc.vector.tensor_copy(out=dst[:, sl], in_=ps)

            def v_init(b):
                nc.vector.memset(vsb[b], 0.0)
                nc.vector.tensor_copy(
                    out=vsb[b][:, :, :, 64:65], in_=ones_col[:, :, :, :]
                )

            def v_transpose(b, h, jc):
                ps = ps_mm.tile([128, 512], FP16, tag="mm", name="ps")
                nc.tensor.transpose(
                    ps[:, 0:64],
                    vt[b][h * 64:(h + 1) * 64, jc * 128:(jc + 1) * 128],
                    ident[h * 64:(h + 1) * 64, h * 64:(h + 1) * 64],
                )
                nc.vector.tensor_copy(out=vsb[b][:, h, jc, 0:64], in_=ps[:, 0:64])

            ones_col = singles.tile([128, 2, 16, 1], FP16, tag="ones")
            nc.vector.memset(ones_col, 1.0)

            # b=0 projections: Q,K first so the softmax exp stream starts
            # as early as possible, then V (+transposes to natural layout)
            # b=0: K fully (scores need all of KT), Q for the first
            # i-block, then V + transposes (PV needs them); Q(it1-3) and
            # all of b=1 are deferred into the attention loop as fillers
            for it in range(4):
                proj_group(0, kt[0], w_sb["wk"], it)
            proj_group(0, qt[0], w_sb["wq"], 0)
            for it in range(4):
                proj_group(0, vt[0], w_sb["wv"], it)
            v_init(0)
            for jc in range(16):
                for h in range(2):
                    v_transpose(0, h, jc)

            # deferred projection work, interleaved into b=0's attention
            # loop to fill PE stalls while ACT (exp) paces it
            filler = []
            for it in range(1, 4):
                filler.append(lambda it=it: proj_group(0, qt[0], w_sb["wq"], it))
            for it in range(4):
                filler.append(lambda it=it: proj_group(1, qt[1], w_sb["wq"], it))
            for it in range(4):
                filler.append(lambda it=it: proj_group(1, kt[1], w_sb["wk"], it))
            for it in range(4):
                filler.append(lambda it=it: proj_group(1, vt[1], w_sb["wv"], it))
            filler.append(lambda: v_init(1))
            for jc in range(16):
                filler.append(lambda jc=jc: (v_transpose(1, 0, jc), v_transpose(1, 1, jc)))

            nonlocal_pace = [0.0]

            def attention(b, emit_filler):
                for it in range(4):
                    i_sl = slice(it * 512, (it + 1) * 512)
                    apv = [
                        ps_apv.tile([128, 512], F32, tag="apv", name=f"apv{_h}")
                        for _h in range(2)
                    ]
                    def score_pair(jc):
                        # one ST tile holds both heads' scores for this jc —
                        # the shared exp forces the two K=64 matmuls adjacent
                        # so they run concurrently in disjoint PE row groups
                        st = ps_mm.tile([128, 1024], F32, tag="mm", name="st")
                        for h in range(2):
                            hs = slice(h * 64, (h + 1) * 64)
                            nc.tensor.matmul(
                                st[:, h * 512:(h + 1) * 512],
                                kt[b][hs, jc * 128:(jc + 1) * 128],
                                qt[b][hs, i_sl],
                                start=True, stop=True,
                            )
                        return st

                    def pv_pair(jc, ptt):
                        for h in range(2):
                            nc.tensor.matmul(
                                apv[h],
                                vsb[b][:, h, jc, :],
                                ptt[:, h * 512:(h + 1) * 512],
                                start=(jc == 0),
                                stop=(jc == 15),
                            )

                    # PV runs one jc behind exp so its weight load never
                    # waits on an in-flight exp
                    sts = {0: score_pair(0)}
                    if 1 < 16:
                        sts[1] = score_pair(1)
                    ptts = {}
                    for jc in range(16):
                        ptts[jc] = pt_sb.tile([128, 1024], FP16, tag="pt", name="pt")
                        nc.scalar.activation(
                            out=ptts[jc], in_=sts.pop(jc), func=AF.Exp,
                            scale=1.0 / (DQ ** 0.5), bias=exp_bias,
                        )
                        if jc + 2 < 16:
                            sts[jc + 2] = score_pair(jc + 2)
                        if jc >= 1:
                            pv_pair(jc - 1, ptts.pop(jc - 1))
                        if emit_filler:
                            nonlocal_pace[0] += 35.0 / 64.0
                            while filler and nonlocal_pace[0] >= 1.0:
                                nonlocal_pace[0] -= 1.0
                                filler.pop(0)()
                    pv_pair(15, ptts.pop(15))
                    for h in range(2):
                        o_sb = out_sb.tile([65, 512], F32, tag="o")
                        nc.vector.tensor_copy(out=o_sb, in_=apv[h][0:65, :])
                        nc.sync.dma_start(
                            out=at_d[b, h * 64:(h + 1) * 64, i_sl], in_=o_sb[0:64, :]
                        )
                        nc.sync.dma_start(
                            out=at_d[b, 128 + h:129 + h, i_sl], in_=o_sb[64:65, :]
                        )

            attention(0, True)
            while filler:
                filler.pop(0)()
            attention(1, False)
    nc.compile()
    return nc


def _build_phase_b():
    nc = bacc.Bacc("TRN2", target_bir_lowering=False, debug=False, num_devices=N_CORES)
    ROWS = B * L // N_CORES  # 512
    atq_d = nc.dram_tensor("atq", [H * DQ, ROWS], FP16, kind="ExternalInput").ap()
    rdn_d = nc.dram_tensor("rdn", [H * DQ, ROWS], FP16, kind="ExternalInput").ap()
    xr_d = nc.dram_tensor("xr", [ROWS, D], F32, kind="ExternalInput").ap()
    wo_d = nc.dram_tensor("wo", [H * DQ, D], FP16, kind="ExternalInput").ap()
    g_d = nc.dram_tensor("gamma", [D], F32, kind="ExternalInput").ap()
    bt_d = nc.dram_tensor("beta", [D], F32, kind="ExternalInput").ap()
    y_d = nc.dram_tensor("y", [ROWS, D], F32, kind="ExternalOutput").ap()

    with tile.TileContext(nc) as tc:
        with tc.tile_pool(name="sb", bufs=1) as sb, \
             tc.tile_pool(name="yt_sb", bufs=3) as yt_sb, \
             tc.tile_pool(name="st_sb", bufs=4) as st_sb, \
             tc.tile_pool(name="ps", bufs=4, space="PSUM") as ps_pool:
            atq = sb.tile([128, 8, ROWS], FP16, tag="atq")
            nc.sync.dma_start(out=atq, in_=atq_d.rearrange("(hc p) i -> p hc i", p=128))
            rdn = sb.tile([128, 8, ROWS], FP16, tag="rdn")
            nc.sync.dma_start(out=rdn, in_=rdn_d.rearrange("(hc p) i -> p hc i", p=128))
            atn = sb.tile([128, 8, ROWS], FP16, tag="atn")
            nc.vector.tensor_tensor(out=atn, in0=atq, in1=rdn, op=OP.mult)
            wo = sb.tile([128, 8, D], FP16, tag="wo")
            nc.sync.dma_start(out=wo, in_=wo_d.rearrange("(hc p) m -> p hc m", p=128))
            x_sb = sb.tile([128, 4, D], F32, tag="x")
            nc.sync.dma_start(out=x_sb, in_=xr_d.rearrange("(ic p) m -> p ic m", p=128))
            gb = sb.tile([128, D], F32, tag="gb")
            nc.sync.dma_start(
                out=gb,
                in_=bass.AP(tensor=g_d.tensor, offset=g_d.offset, ap=[[0, 128]] + g_d.ap),
            )
            bb = sb.tile([128, D], F32, tag="bb")
            nc.sync.dma_start(
                out=bb,
                in_=bass.AP(tensor=bt_d.tensor, offset=bt_d.offset, ap=[[0, 128]] + bt_d.ap),
            )
            eps_t = sb.tile([128, 1], F32, tag="eps")
            nc.vector.memset(eps_t, LN_EPS)

            for ic in range(4):
                yt = yt_sb.tile([128, D], F32, tag="yt")
                for mh in range(2):
                    o_ps = ps_pool.tile([128, 512], F32, tag="o")
                    for hc in range(8):
                        nc.tensor.matmul(
                            o_ps,
                            atn[:, hc, ic * 128:(ic + 1) * 128],
                            wo[:, hc, mh * 512:(mh + 1) * 512],
                            start=(hc == 0), stop=(hc == 7),
                        )
                    nc.vector.tensor_tensor(
                        out=yt[:, mh * 512:(mh + 1) * 512],
                        in0=o_ps,
                        in1=x_sb[:, ic, mh * 512:(mh + 1) * 512],
                        op=OP.add,
                    )
                stats = st_sb.tile([128, 2, 6], F32, tag="stats")
                for sg in range(2):
                    nc.vector.bn_stats(
                        out=stats[:, sg, :], in_=yt[:, sg * 512:(sg + 1) * 512]
                    )
                mv = st_sb.tile([128, 2], F32, tag="mv")
                nc.vector.bn_aggr(out=mv, in_=stats)
                rstd = st_sb.tile([128, 1], F32, tag="rstd")
                nc.scalar.activation(
                    out=rstd, in_=mv[:, 1:2], func=AF.Sqrt, bias=eps_t, scale=1.0
                )
                nc.vector.reciprocal(out=rstd, in_=rstd)
                nc.vector.tensor_scalar(
                    out=yt, in0=yt, scalar1=mv[:, 0:1], scalar2=rstd,
                    op0=OP.subtract, op1=OP.mult,
                )
                nc.vector.tensor_tensor(out=yt, in0=yt, in1=gb, op=OP.mult)
                nc.vector.tensor_tensor(out=yt, in0=yt, in1=bb, op=OP.add)
                nc.sync.dma_start(out=y_d[ic * 128:(ic + 1) * 128, :], in_=yt)
    nc.compile()
    return nc


def _prep_a(x, w_q, w_k, w_v):
    xt = np.ascontiguousarray(x.transpose(0, 2, 1)).astype(FP16_NP)  # [B, D, L]

    def w_slice(w, c):
        return np.ascontiguousarray(
            w[2 * c:2 * c + 2].transpose(1, 0, 2).reshape(D, 2 * DQ)
        ).astype(FP16_NP)

    return [
        {
            "xt": xt,
            "wq": w_slice(w_q, c),
            "wk": w_slice(w_k, c),
            "wv": w_slice(w_v, c),
        }
        for c in range(N_CORES)
    ]


def _prep_b(res_a_results, x, w_o, ln_gamma, ln_beta):
    at_full = np.concatenate(
        [res_a_results[c]["at"][:, :128, :] for c in range(N_CORES)], axis=1
    )  # [B, H*DQ, L]
    den = np.stack(
        [res_a_results[c]["at"][:, 128:130, :] for c in range(N_CORES)], axis=1
    ).reshape(B, H, L)
    rdn_full = np.repeat((1.0 / den).astype(np.float32), DQ, axis=1)  # [B, H*DQ, L]

    ROWS = B * L // N_CORES
    wo_flat = np.ascontiguousarray(w_o.reshape(H * DQ, D)).astype(FP16_NP)
    # exact power-of-two rescale keeps both factors in fp16 range;
    # it cancels exactly in the on-device product
    at_bf = (at_full * (1.0 / 64.0)).astype(FP16_NP)
    rdn_bf = (rdn_full * 64.0).astype(FP16_NP)
    in_maps_b = []
    for c in range(N_CORES):
        b = c // (N_CORES // B)
        l0 = (c % (N_CORES // B)) * ROWS
        in_maps_b.append(
            {
                "atq": np.ascontiguousarray(at_bf[b][:, l0:l0 + ROWS]),
                "rdn": np.ascontiguousarray(rdn_bf[b][:, l0:l0 + ROWS]),
                "xr": np.ascontiguousarray(x[b, l0:l0 + ROWS]),
                "wo": wo_flat,
                "gamma": ln_gamma,
                "beta": ln_beta,
            }
        )
    return in_maps_b


def kernel(x, w_q, w_k, w_v, w_o, ln_gamma, ln_beta):
    x = np.asarray(x, dtype=np.float32)
    w_q = np.asarray(w_q, dtype=np.float32)
    w_k = np.asarray(w_k, dtype=np.float32)
    w_v = np.asarray(w_v, dtype=np.float32)
    w_o = np.asarray(w_o, dtype=np.float32)
    ln_gamma = np.asarray(ln_gamma, dtype=np.float32)
    ln_beta = np.asarray(ln_beta, dtype=np.float32)

    if "a" not in _cache:
        _cache["a"] = _build_phase_a()
    if "b" not in _cache:
        _cache["b"] = _build_phase_b()

    in_maps_a = _prep_a(x, w_q, w_k, w_v)
    res_a = run_bass_kernel_spmd(
        _cache["a"], in_maps_a, core_ids=list(range(N_CORES)), trace=False
    )
    in_maps_b = _prep_b(res_a.results, x, w_o, ln_gamma, ln_beta)
    res_b = run_bass_kernel_spmd(
        _cache["b"], in_maps_b, core_ids=list(range(N_CORES)), trace=False
    )
    y = np.concatenate([res_b.results[c]["y"] for c in range(N_CORES)], axis=0)
    return y.reshape(B, L, D)
